# revision 8
# baseline (speedup 1.0000x reference)
# Trainium2 Bass kernel for nn_ExpertLinear (MoE grouped GEMM with routing).
#
# v2.1 strategy (profile-driven fixes over v2):
#   * 4-expert covering design: core c serves expert set BSET[c]; every
#     expert PAIR is inside some core's set, so each token (2 expert slots)
#     is computed and combined entirely on one core, while each core loads
#     only 4 of 8 expert weight matrices.
#   * Host-side dispatch: the routing tables are host-known, so x rows are
#     gathered/transposed into each core's stationary [P, tile, k, 128]
#     layout on the host and DMA'd as plain contiguous tiles.
#   * Same-expert pairs merged: a token routed twice to expert e becomes
#     one GEMM row with gate (g0+g1)/2 and r0 == r1 in the combine.
#   * MEXP[c] (the expert whose merged tokens get the direct-out tile) is
#     pinned to slot 3, so the direct tile reuses the slot-3 weight tile:
#     4 weight loads (8.4 MB) instead of 5 (10.5 MB).
#   * Coarse DMA: x tiles stream per-slot (2KB+ per partition descriptors),
#     prefetched one slot ahead; all 4 weights resident (wpool bufs=4) and
#     issued upfront.  Fixes the v2 startup stall (first 13 us of tensor
#     idle came from ~40 serialized 256B-descriptor DMA issues).
#   * Tail: gather chunks [128,128,96,32]; each chunk's r0-gather fires at
#     its own (earlier) readiness, adds+outs inline; the small final chunk
#     overlaps the direct tile's matmuls.
import os
import numpy as np

import concourse.bacc as bacc
import concourse.bass as bass
import concourse.mybir as mybir
import concourse.tile as tile
from concourse.bass_utils import run_bass_kernel_spmd

N_TOK = 8192
TOPK = 2
N_EXP = 8
D_IN = 1024
D_OUT = 1024
NCORES = 8
TPC = N_TOK // NCORES
P = 128
KT = D_IN // P
NSLOT = 4
F16 = mybir.dt.float16
F32 = mybir.dt.float32
I16 = mybir.dt.int16

# Covering design: every unordered expert pair {a,b} (incl. a==b) is a
# subset of at least one block; each expert appears in exactly 4 blocks.
BLOCKS = [(0, 1, 2, 3), (4, 5, 6, 7), (0, 1, 4, 5), (2, 3, 6, 7),
          (0, 2, 4, 6), (1, 3, 5, 7), (0, 3, 4, 7), (1, 2, 5, 6)]
BSET = [frozenset(blk) for blk in BLOCKS]

CHUNKS = [128, 128, 128]                 # gather-combine chunk sizes
NGATH = sum(CHUNKS)                      # gather-combine token slots
NDIR = P                                 # merged direct-out token slots

# Aligned-combine tile pairs (slot_a, slot_b, k): the k-th tile of slot_a
# and of slot_b form a DVE-added pair; cross tokens of the expert-pair
# type mapped to (slot_a, slot_b) sit at MATCHING positions in both tiles.
APAIRS = [(0, 1, 0), (2, 3, 0), (0, 2, 1), (1, 3, 1),
          (0, 3, 2), (1, 2, 2), (0, 1, 3), (2, 3, 3)]
NAL = len(APAIRS)                        # aligned out blocks (P tokens each)
NOUT = NAL * P + NDIR + NGATH            # out slots per core

# core -> expert whose merged tokens get this core's direct-out tile
# (bijection, e in BLOCKS[c]); pinned to slot 3 so the direct tile can
# reuse the slot-3 weight tile.
MEXP = [0, 4, 1, 2, 6, 3, 7, 5]


def _pack16(flat):
    # [16, n/16] block (idx j at [j%16, j//16]), replicated into all eight
    # 16-partition groups — each GpSimd Q7 core reads its own copy.
    return np.ascontiguousarray(np.tile(flat.reshape(-1, 16).T, (8, 1)))


def _cores_of_pair():
    m = {}
    for x in range(N_EXP):
        for y in range(x, N_EXP):
            m[x * N_EXP + y] = [c for c in range(NCORES)
                                if x in BSET[c] and y in BSET[c]]
    return m


def _assign_counts(cnt_of):
    """Distribute gather-path tokens (by pair group) over covering cores.
    cnt_of: pair id -> token count (cross pairs + overflow merged).
    Returns n[pid][core] counts, balancing per-core loads and
    per-(core, expert) row counts toward <= 512."""
    cop = _cores_of_pair()
    load = np.zeros(NCORES, np.int64)
    rcnt = np.zeros((NCORES, N_EXP), np.int64)
    n = {p: np.zeros(NCORES, np.int64) for p in cnt_of}

    def rows_of(p):
        a, b = p // N_EXP, p % N_EXP
        return (a, b) if a != b else (a,)

    groups = sorted(cnt_of, key=lambda p: (len(cop[p]), -cnt_of[p]))
    for p in groups:
        cs = cop[p]
        rexp = rows_of(p)
        for _ in range(cnt_of[p]):
            best, bc = None, None
            for c in cs:
                if load[c] >= NAL * P + NGATH:
                    continue
                cost = 4.0 * load[c] + sum(rcnt[c, e] for e in rexp)
                if best is None or cost < best:
                    best, bc = cost, c
            assert bc is not None, "balancer stuck"
            n[p][bc] += 1
            load[bc] += 1
            for e in rexp:
                rcnt[bc, e] += 1

    # quadratic-potential refinement
    WL = 0.3
    loads = load
    for _ in range(40):
        improved = False
        for p in groups:
            cs = cop[p]
            if len(cs) < 2:
                continue
            exps = rows_of(p)
            for cf in cs:
                for ct in cs:
                    if ct == cf:
                        continue
                    while n[p][cf] > 0 and loads[ct] < NAL * P + NGATH:
                        d = WL * 2.0 * (loads[ct] - loads[cf] + 1)
                        for e in exps:
                            d += 2.0 * (rcnt[ct, e] - rcnt[cf, e] + 1)
                        if d >= 0:
                            break
                        n[p][cf] -= 1
                        n[p][ct] += 1
                        loads[cf] -= 1
                        loads[ct] += 1
                        for e in exps:
                            rcnt[cf, e] -= 1
                            rcnt[ct, e] += 1
                        improved = True
        if not improved:
            break
    return n, rcnt


def _plan(te, tg):
    """Host routing plan.  te [N_TOK, 2] expert ids, tg [N_TOK, 2] gates.
    Returns (T, chunk readiness, per_core input dicts, token id tables)."""
    a = np.minimum(te[:, 0], te[:, 1])
    b = np.maximum(te[:, 0], te[:, 1])
    pid = (a * N_EXP + b).astype(np.int64)
    merged = a == b
    gsum = tg.sum(axis=1)

    direct_toks = [None] * NCORES
    gather_tok = np.ones(N_TOK, bool)
    for c in range(NCORES):
        e = MEXP[c]
        toks = np.where(merged & (a == e))[0][:NDIR]
        direct_toks[c] = toks
        gather_tok[toks] = False

    cnt_of = {}
    for p in np.unique(pid[gather_tok]):
        cnt_of[int(p)] = int(((pid == p) & gather_tok).sum())
    n, rcnt = _assign_counts(cnt_of)

    core_of = np.full(N_TOK, -1, np.int64)
    for p, npc in n.items():
        toks = np.where((pid == p) & gather_tok)[0]
        base = 0
        for c in range(NCORES):
            k = int(npc[c])
            core_of[toks[base:base + k]] = c
            base += k
    assert (core_of[gather_tok] >= 0).all()

    # per-core expert -> slot; MEXP[c] pinned to slot 3, brute-force over
    # the other 3 experts minimizing weighted aligned-pair overflow
    import itertools
    pair_caps = {}
    for sa, sb, k in APAIRS:
        pair_caps.setdefault((sa, sb), []).append(k)
    slots = []
    for c in range(NCORES):
        toks_c = np.where(core_of == c)[0]
        tcnt = {}
        movf = {}
        for t in toks_c:
            ea, eb = int(te[t, 0]), int(te[t, 1])
            if ea != eb:
                tcnt[(min(ea, eb), max(ea, eb))] = \
                    tcnt.get((min(ea, eb), max(ea, eb)), 0) + 1
            else:
                movf[ea] = movf.get(ea, 0) + 1
        rest = [e for e in BLOCKS[c] if e != MEXP[c]]
        best, bperm = None, None
        for perm3 in itertools.permutations(rest):
            perm = tuple(perm3) + (MEXP[c],)
            sl = {e: s for s, e in enumerate(perm)}
            exc_w, exc_tot = 0.0, 0
            for (ea, eb), cnt in tcnt.items():
                sa, sb = sorted((sl[ea], sl[eb]))
                cap = P * len(pair_caps.get((sa, sb), []))
                ex = max(0, cnt - cap)
                exc_tot += ex
                exc_w += ex * (1.0 + 4.0 * sb)
            for e, cnt in movf.items():
                exc_w += 0.5 * cnt * (1.0 + 2.0 * sl[e])
            score = exc_w + 1e6 * max(0, exc_tot - NGATH)
            if best is None or score < best:
                best, bperm = score, perm
        slots.append(list(bperm))

    cnt_cs = np.zeros((NCORES, NSLOT), np.int64)
    for c in range(NCORES):
        for s, e in enumerate(slots[c]):
            cnt_cs[c, s] = rcnt[c, e]
    T = np.maximum(1, -(-cnt_cs.max(axis=0) // P))
    off_t = np.concatenate([[0], np.cumsum(T)])
    NBs = int(T.sum())
    NB = NBs + 1
    NP = NB * P
    apairs = [(sa, sb, k, int(off_t[sa]) + k, int(off_t[sb]) + k)
              for sa, sb, k in APAIRS if k < T[sa] and k < T[sb]]

    per_core = []
    token_ids = []
    ready0_all = np.zeros((NCORES, len(CHUNKS)), np.int64)
    ready1_all = np.zeros((NCORES, len(CHUNKS)), np.int64)
    for c in range(NCORES):
        slot_of = {e: s for s, e in enumerate(slots[c])}
        toks_c = np.where(core_of == c)[0]
        by_sp = {}
        gather_list = []
        for t in toks_c:
            ea, eb = int(te[t, 0]), int(te[t, 1])
            if ea == eb:
                gather_list.append(t)
            else:
                sa, sb = sorted((slot_of[ea], slot_of[eb]))
                by_sp.setdefault((sa, sb), []).append(t)
        grow_flat = np.zeros(NP, np.float32)
        src_tok = np.full(NP, -1, np.int64)
        fill = np.zeros(NBs, np.int64)
        amap = np.full(NAL * P, -1, np.int64)
        alloc_order = sorted(range(len(apairs)),
                             key=lambda i: -apairs[i][2])
        for j in alloc_order:
            (sa, sb, k, ga, gb) = apairs[j]
            lst = by_sp.get((sa, sb), [])
            nal = min(len(lst), P)
            for p in range(nal):
                t = lst[p]
                amap[j * P + p] = t
                ea, eb = int(te[t, 0]), int(te[t, 1])
                if slot_of[ea] != sa:
                    ea, eb = eb, ea
                    g0, g1 = float(tg[t, 1]), float(tg[t, 0])
                else:
                    g0, g1 = float(tg[t, 0]), float(tg[t, 1])
                ra, rb = ga * P + p, gb * P + p
                grow_flat[ra] = g0
                src_tok[ra] = t
                grow_flat[rb] = g1
                src_tok[rb] = t
            by_sp[(sa, sb)] = lst[nal:]
            fill[ga] = max(fill[ga], nal)
            fill[gb] = max(fill[gb], nal)
        for lst in by_sp.values():
            gather_list.extend(lst)

        row_of = {}
        free = {int(g): list(range(int(fill[g]), P)) for g in range(NBs)}

        def place_row(s, t, g_val):
            for g in range(NBs):
                if off_t[s] <= g < off_t[s + 1] and free[g]:
                    p_ = free[g].pop(0)
                    r = g * P + p_
                    grow_flat[r] = g_val
                    src_tok[r] = t
                    row_of.setdefault(t, []).append(r)
                    return
            raise AssertionError("no free row slot")

        for t in gather_list:
            ea, eb = int(te[t, 0]), int(te[t, 1])
            if ea == eb:
                place_row(slot_of[ea], t, float(gsum[t]) * 0.5)
            else:
                place_row(slot_of[ea], t, float(tg[t, 0]))
                place_row(slot_of[eb], t, float(tg[t, 1]))

        for i, t in enumerate(direct_toks[c]):
            r = NBs * P + i
            grow_flat[r] = gsum[t]
            src_tok[r] = t

        # sort each token's rows by tile so r0 is the earlier-ready one
        for t in row_of:
            row_of[t].sort(key=lambda r: r // P)
        ready = np.array([row_of[t][-1] // P for t in gather_list], np.int64)
        order = np.argsort(ready, kind="stable")
        gl = np.array(gather_list, np.int64)[order]
        npad = NGATH - len(gl)
        assert npad >= 0, (c, len(gl))
        gtoks = np.concatenate([np.full(npad, -1, np.int64), gl])
        r0_flat = np.zeros(NGATH, np.int16)
        r1_flat = np.zeros(NGATH, np.int16)
        rd0 = np.zeros(NGATH, np.int64)
        rd1 = np.zeros(NGATH, np.int64)
        for pos, t in enumerate(gtoks):
            if t < 0:
                continue
            rs = row_of[t]
            r0_flat[pos] = rs[0]
            r1_flat[pos] = rs[-1] if len(rs) > 1 else rs[0]
            rd0[pos] = r0_flat[pos] // P
            rd1[pos] = r1_flat[pos] // P
        bounds = np.cumsum(CHUNKS)
        for ci in range(len(CHUNKS)):
            lo = bounds[ci] - CHUNKS[ci]
            ready0_all[c, ci] = rd0[lo:bounds[ci]].max()
            ready1_all[c, ci] = rd1[lo:bounds[ci]].max()
        dpad = np.full(NDIR, -1, np.int64)
        dpad[:len(direct_toks[c])] = direct_toks[c]
        token_ids.append((amap, dpad, gtoks))

        per_core.append(dict(
            grow=np.ascontiguousarray(grow_flat.reshape(NB, P).T),
            r0i=_pack16(r0_flat),
            r1i=_pack16(r1_flat),
            src_tok=src_tok,
            slot_experts=np.array(slots[c]),
        ))

    RT0 = np.maximum.accumulate(ready0_all.max(axis=0))
    RT1 = np.maximum.accumulate(ready1_all.max(axis=0))
    RT0 = np.minimum(RT0, RT1)
    max_real = max(int((tid[2] >= 0).sum()) for tid in token_ids)
    bounds = np.cumsum(CHUNKS)
    for ci in range(len(CHUNKS)):
        if bounds[ci] <= NGATH - max_real:
            RT1[ci] = -1                 # all-pad chunk: skip
    return T, RT0, RT1, per_core, token_ids


def _build_nc(T, RT0, RT1):
    NBs = int(T.sum())
    NB = NBs + 1
    off_tiles = np.concatenate([[0], np.cumsum(T)])

    nc = bacc.Bacc("TRN2", target_bir_lowering=False, debug=False,
                   num_devices=NCORES)

    xg = nc.dram_tensor("xg", [P, NB, KT, P], F16, kind="ExternalInput")
    wh = nc.dram_tensor("wh", [NSLOT, P, KT, D_OUT], F16,
                        kind="ExternalInput")
    grow = nc.dram_tensor("grow", [P, NB], F32, kind="ExternalInput")
    r0i = nc.dram_tensor("r0i", [P, NGATH // 16], I16, kind="ExternalInput")
    r1i = nc.dram_tensor("r1i", [P, NGATH // 16], I16, kind="ExternalInput")
    xwarm = nc.dram_tensor("xwarm", [P, P], F16, kind="ExternalInput")
    outT = nc.dram_tensor("outT", [P, (D_OUT // P) * NOUT], F16,
                          kind="ExternalOutput")

    # Pre-TileContext warmup: the first DMAGatherAnt triggers a ~15us Q7
    # extended-instruction library fetch; start it ASAP so it overlaps
    # the input DMAs and the first matmul tiles.
    warm_idx = nc.alloc_sbuf_tensor("warm_idx", [P, 8], I16)
    warm_dst = nc.alloc_sbuf_tensor("warm_dst", [P, P], F16)
    warm_sem = nc.alloc_semaphore("warm_set")
    warm_dma = nc.alloc_semaphore("warm_dma")
    nc.gpsimd.memset(warm_idx.ap(), 0).then_inc(warm_sem, 1)
    nc.gpsimd.wait_ge(warm_sem, 1)
    nc.gpsimd.dma_gather(
        warm_dst.ap().rearrange("p (a b) -> p a b", a=1),
        xwarm[:].rearrange("n (a b) -> (n a) b", b=P),
        warm_idx.ap(), num_idxs=P, num_idxs_reg=P, elem_size=P,
        transpose=True).then_inc(warm_dma, 16)

    # chunk schedule: (chunk idx, token base, size, r0-ready, r1-ready);
    # RT1 == -1 marks an all-pad chunk (not emitted)
    chunk_q = []
    base = 0
    for ci, ch in enumerate(CHUNKS):
        if int(RT1[ci]) >= 0:
            chunk_q.append([ci, base, ch, int(RT0[ci]), int(RT1[ci])])
        base += ch
    apairs = [(j, int(off_tiles[sa]) + k, int(off_tiles[sb]) + k)
              for j, (sa, sb, k) in enumerate(APAIRS)
              if k < T[sa] and k < T[sb]]

    with tile.TileContext(nc) as tc:
        with (
            tc.tile_pool(name="const", bufs=1) as kpool,
            tc.tile_pool(name="w", bufs=4) as wpool,
            tc.tile_pool(name="xT", bufs=1) as xpool,
            tc.tile_pool(name="y", bufs=1) as ypool,
            tc.tile_pool(name="cmb", bufs=8) as cpool,
            tc.tile_pool(name="ot", bufs=3) as opool,
            tc.tile_pool(name="ps", bufs=6, space="PSUM") as ppool,
        ):
            x_t = xpool.tile([P, NB, KT, P], F16)
            # critical path: w0 k0 + x tile0 k0 first, then coarse bulk
            w_ts = [wpool.tile([P, KT, D_OUT], F16, tag="w",
                               name="w%d" % s_) for s_ in range(NSLOT)]
            nc.scalar.dma_start(w_ts[0][:, 0], wh[0, :, 0])
            nc.sync.dma_start(x_t[:, 0, 0], xg[:, 0, 0])
            nc.scalar.dma_start(w_ts[0][:, 1:KT], wh[0, :, 1:KT])
            nc.sync.dma_start(x_t[:, 0, 1:KT], xg[:, 0, 1:KT])
            if off_tiles[1] > 1:
                nc.sync.dma_start(x_t[:, 1:off_tiles[1]],
                                  xg[:, 1:off_tiles[1]])
            grow_t = kpool.tile([P, NB], F32)
            nc.sync.dma_start(grow_t[:], grow[:])
            # remaining weights upfront (all resident; queue is FIFO so
            # they stream in slot order behind w0)
            for s in range(1, NSLOT):
                nc.scalar.dma_start(w_ts[s][:], wh[s])
            r0_t = kpool.tile([P, NGATH // 16], I16)
            nc.sync.dma_start(r0_t[:], r0i[:])
            r1_t = kpool.tile([P, NGATH // 16], I16)
            nc.sync.dma_start(r1_t[:], r1i[:])

            y_t = ypool.tile([P, NB, D_OUT], F16)

            gtiles = {}                  # ci -> (g0, g1, cbase, csize)

            def emit_gather(ci, cbase, csize, rt, which):
                dst = cpool.tile([P, D_OUT // P, csize], F16,
                                 tag="g%d_%d" % (which, csize), bufs=2)
                ridx = r0_t if which == 0 else r1_t
                nc.gpsimd.dma_gather(
                    dst[:], y_t[:, :rt + 1, :],
                    ridx[:, cbase // 16:(cbase + csize) // 16],
                    num_idxs=csize, num_idxs_reg=csize,
                    elem_size=D_OUT, transpose=True,
                    sbuf_tokens_per_rank=P,
                    sbuf_free_dim_per_rank=D_OUT * 2,
                )
                return dst

            gbase = (NAL * P + NDIR) * (D_OUT // P)

            def emit_chunk_out(ci, cbase, csize):
                g0, g1 = gtiles[ci]
                ot = opool.tile([P, D_OUT // P, csize], F16,
                                tag="ots_%d" % csize, bufs=2)
                nc.vector.tensor_add(out=ot[:], in0=g0[:], in1=g1[:])
                nc.sync.dma_start(
                    outT[:, gbase + cbase * (D_OUT // P):
                         gbase + (cbase + csize) * (D_OUT // P)],
                    ot[:].rearrange("p a b -> p (a b)"))

            qi = 0          # next chunk awaiting full emission (r1 + out)
            q0 = 0          # next chunk awaiting r0-gather emission

            def pump_chunks(g):
                nonlocal qi, q0
                while q0 < len(chunk_q) and chunk_q[q0][3] <= g:
                    ci, cbase, csize, rt0, rt1 = chunk_q[q0]
                    gtiles[ci] = [emit_gather(ci, cbase, csize, rt0, 0),
                                  None]
                    q0 += 1
                while qi < len(chunk_q) and chunk_q[qi][4] <= g:
                    ci, cbase, csize, rt0, rt1 = chunk_q[qi]
                    if qi >= q0:         # r0 not yet emitted (rt0 race)
                        gtiles[ci] = [emit_gather(ci, cbase, csize, rt1, 0),
                                      None]
                        q0 = qi + 1
                    gtiles[ci][1] = emit_gather(ci, cbase, csize, rt1, 1)
                    emit_chunk_out(ci, cbase, csize)
                    qi += 1

            for s in range(NSLOT + 1):
                w_t = w_ts[min(s, NSLOT - 1)]
                # prefetch next slot's x one slot ahead
                if s < NSLOT:
                    nlo = int(off_tiles[s + 1]) if s + 1 < NSLOT else NBs
                    nhi = int(off_tiles[s + 2]) if s + 1 < NSLOT else NB
                    if s + 1 == NSLOT:
                        nlo, nhi = NBs, NB
                    nc.sync.dma_start(x_t[:, nlo:nhi], xg[:, nlo:nhi])
                lo = int(off_tiles[s]) if s < NSLOT else NBs
                hi = int(off_tiles[s + 1]) if s < NSLOT else NB
                for g in range(lo, hi):
                    ps0 = ppool.tile([P, 512], F32, tag="ps")
                    ps1 = ppool.tile([P, 512], F32, tag="ps")
                    for kk in range(KT):
                        lhsT = x_t[:, g, kk, :]
                        nc.tensor.matmul(ps0[:], lhsT, w_t[:, kk, 0:512],
                                         start=(kk == 0), stop=(kk == KT - 1))
                        nc.tensor.matmul(ps1[:], lhsT, w_t[:, kk, 512:1024],
                                         start=(kk == 0), stop=(kk == KT - 1))
                    gsc = grow_t[:, g:g + 1]
                    nc.scalar.activation(y_t[:, g, 0:512], ps0[:],
                                         mybir.ActivationFunctionType.Copy,
                                         scale=gsc)
                    nc.scalar.activation(y_t[:, g, 512:1024], ps1[:],
                                         mybir.ActivationFunctionType.Copy,
                                         scale=gsc)
                    for j, ga, gb in apairs:
                        if max(ga, gb) != g:
                            continue
                        ot = opool.tile([P, D_OUT], F16, tag="al")
                        nc.vector.tensor_add(out=ot[:], in0=y_t[:, ga, :],
                                             in1=y_t[:, gb, :])
                        nc.sync.dma_start(
                            outT[:, j * P * (D_OUT // P):
                                 (j + 1) * P * (D_OUT // P)],
                            ot[:])
                    if g < NBs:
                        pump_chunks(g)
            pump_chunks(NBs)
            # merged direct-out tile: evicted y rows ARE output rows
            nc.scalar.dma_start(
                outT[:, NAL * P * (D_OUT // P):
                     (NAL * P + NDIR) * (D_OUT // P)],
                y_t[:, NB - 1, :])

    nc.compile()
    return nc


def _prep(inputs):
    x = np.asarray(inputs["input"], np.float32)
    w = np.asarray(inputs["weight"], np.float32)
    k = int(np.asarray(inputs["k"]))
    assert k == TOPK
    sei = np.asarray(inputs["sorted_expert_indices"]).astype(np.int64)
    ssi = np.asarray(inputs["sorted_scattered_indices"]).astype(np.int64)
    gates = np.asarray(inputs["gates"], np.float32)

    tok = ssi // k
    g_row = gates.reshape(-1)[ssi]
    order_by_tok = np.argsort(tok, kind="stable")
    te = sei[order_by_tok].reshape(N_TOK, TOPK)
    tg = g_row[order_by_tok].reshape(N_TOK, TOPK)

    T, RT0, RT1, per_core, token_ids = _plan(te, tg)
    NB = int(T.sum()) + 1
    NP = NB * P

    xh = x.astype(np.float16)
    whp = np.ascontiguousarray(
        w.reshape(N_EXP, KT, P, D_OUT).transpose(0, 2, 1, 3)
    ).astype(np.float16)               # [E, P, KT, D_OUT]
    xwarm = np.zeros((P, P), np.float16)

    in_maps = []
    for c in range(NCORES):
        pc = per_core[c]
        src = pc.pop("src_tok")
        se = pc.pop("slot_experts")
        A = np.zeros((NP, D_IN), np.float16)
        m = src >= 0
        A[m] = xh[src[m]]
        AT = np.ascontiguousarray(A.T)                  # [D_IN, NP]
        xgc = np.ascontiguousarray(
            AT.reshape(KT, P, NB, P).transpose(1, 2, 0, 3))
        in_maps.append(dict(
            xg=xgc,
            wh=np.ascontiguousarray(whp[se]),
            grow=pc["grow"], r0i=pc["r0i"], r1i=pc["r1i"],
            xwarm=xwarm,
        ))
    return T, RT0, RT1, in_maps, token_ids


def _run(inputs, trace=False, trace_kwargs=None):
    T, RT0, RT1, in_maps, token_ids = _prep(inputs)
    nc = _build_nc(T, RT0, RT1)
    res = run_bass_kernel_spmd(
        nc, in_maps, core_ids=list(range(NCORES)), trace=trace,
        **(trace_kwargs or {}),
    )
    out = np.zeros((N_TOK, D_OUT), np.float32)
    for c in range(NCORES):
        oT = res.results[c]["outT"]                      # [P, 8*NOUT]
        amap, dtoks, gtoks = token_ids[c]
        ablk = oT[:, :(NAL * P + NDIR) * (D_OUT // P)]
        rows = ablk.reshape(P, NAL + 1, D_OUT).transpose(1, 0, 2) \
                   .reshape((NAL + 1) * P, D_OUT)
        ids = np.concatenate([amap, dtoks])
        m = ids >= 0
        out[ids[m]] = rows[m]
        gbase = (NAL * P + NDIR) * (D_OUT // P)
        base = 0
        for ch in CHUNKS:
            blk = oT[:, gbase + base * (D_OUT // P):
                     gbase + (base + ch) * (D_OUT // P)]
            rws = blk.reshape(P, D_OUT // P, ch).transpose(2, 1, 0)
            ids = gtoks[base:base + ch]
            m = ids >= 0
            out[ids[m]] = rws.reshape(ch, D_OUT)[m]
            base += ch
    return out, res


def kernel(**inputs) -> np.ndarray:
    out, _ = _run(inputs, trace=bool(int(os.environ.get("KERNEL_TRACE", "0"))))
    return out


# revision 16
# speedup vs baseline: 1.0197x; 1.0197x over previous
# Trainium2 Bass kernel for nn_ExpertLinear (MoE grouped GEMM with routing).
#
# v2.1 strategy (profile-driven fixes over v2):
#   * 4-expert covering design: core c serves expert set BSET[c]; every
#     expert PAIR is inside some core's set, so each token (2 expert slots)
#     is computed and combined entirely on one core, while each core loads
#     only 4 of 8 expert weight matrices.
#   * Host-side dispatch: the routing tables are host-known, so x rows are
#     gathered/transposed into each core's stationary [P, tile, k, 128]
#     layout on the host and DMA'd as plain contiguous tiles.
#   * Same-expert pairs merged: a token routed twice to expert e becomes
#     one GEMM row with gate (g0+g1)/2 and r0 == r1 in the combine.
#   * MEXP[c] (the expert whose merged tokens get the direct-out tile) is
#     pinned to slot 3, so the direct tile reuses the slot-3 weight tile:
#     4 weight loads (8.4 MB) instead of 5 (10.5 MB).
#   * Coarse DMA: x tiles stream per-slot (2KB+ per partition descriptors),
#     prefetched one slot ahead; all 4 weights resident (wpool bufs=4) and
#     issued upfront.  Fixes the v2 startup stall (first 13 us of tensor
#     idle came from ~40 serialized 256B-descriptor DMA issues).
#   * Tail: gather chunks [128,128,96,32]; each chunk's r0-gather fires at
#     its own (earlier) readiness, adds+outs inline; the small final chunk
#     overlaps the direct tile's matmuls.
import os
import numpy as np

import concourse.bacc as bacc
import concourse.bass as bass
import concourse.mybir as mybir
import concourse.tile as tile
from concourse.bass_utils import run_bass_kernel_spmd

N_TOK = 8192
TOPK = 2
N_EXP = 8
D_IN = 1024
D_OUT = 1024
NCORES = 8
TPC = N_TOK // NCORES
P = 128
KT = D_IN // P
NSLOT = 4
F16 = mybir.dt.float16
F32 = mybir.dt.float32
I16 = mybir.dt.int16

# Covering design: every unordered expert pair {a,b} (incl. a==b) is a
# subset of at least one block; each expert appears in exactly 4 blocks.
BLOCKS = [(0, 1, 2, 3), (4, 5, 6, 7), (0, 1, 4, 5), (2, 3, 6, 7),
          (0, 2, 4, 6), (1, 3, 5, 7), (0, 3, 4, 7), (1, 2, 5, 6)]
BSET = [frozenset(blk) for blk in BLOCKS]

CHUNKS = [128, 128, 128]                 # gather-combine chunk sizes
NGATH = sum(CHUNKS)                      # gather-combine token slots
NDIR = P                                 # merged direct-out token slots

# Aligned-combine tile pairs (slot_a, slot_b, k): the k-th tile of slot_a
# and of slot_b form a DVE-added pair; cross tokens of the expert-pair
# type mapped to (slot_a, slot_b) sit at MATCHING positions in both tiles.
APAIRS = [(0, 1, 0), (2, 3, 0), (0, 2, 1), (1, 3, 1),
          (0, 3, 2), (1, 2, 2), (0, 1, 3), (2, 3, 3)]
NAL = len(APAIRS)                        # aligned out blocks (P tokens each)
NOUT = NAL * P + NDIR + NGATH            # out slots per core

# core -> expert whose merged tokens get this core's direct-out tile
# (bijection, e in BLOCKS[c]); pinned to slot 3 so the direct tile can
# reuse the slot-3 weight tile.
MEXP = [0, 4, 1, 2, 6, 3, 7, 5]


def _pack16(flat):
    # [16, n/16] block (idx j at [j%16, j//16]), replicated into all eight
    # 16-partition groups — each GpSimd Q7 core reads its own copy.
    return np.ascontiguousarray(np.tile(flat.reshape(-1, 16).T, (8, 1)))


def _cores_of_pair():
    m = {}
    for x in range(N_EXP):
        for y in range(x, N_EXP):
            m[x * N_EXP + y] = [c for c in range(NCORES)
                                if x in BSET[c] and y in BSET[c]]
    return m


def _assign_counts(cnt_of):
    """Distribute gather-path tokens (by pair group) over covering cores.
    cnt_of: pair id -> token count (cross pairs + overflow merged).
    Returns n[pid][core] counts, balancing per-core loads and
    per-(core, expert) row counts toward <= 512."""
    cop = _cores_of_pair()
    load = np.zeros(NCORES, np.int64)
    rcnt = np.zeros((NCORES, N_EXP), np.int64)
    n = {p: np.zeros(NCORES, np.int64) for p in cnt_of}

    def rows_of(p):
        a, b = p // N_EXP, p % N_EXP
        return (a, b) if a != b else (a,)

    groups = sorted(cnt_of, key=lambda p: (len(cop[p]), -cnt_of[p]))
    for p in groups:
        cs = cop[p]
        rexp = rows_of(p)
        for _ in range(cnt_of[p]):
            best, bc = None, None
            for c in cs:
                if load[c] >= NAL * P + NGATH:
                    continue
                cost = 4.0 * load[c] + sum(rcnt[c, e] for e in rexp)
                if best is None or cost < best:
                    best, bc = cost, c
            assert bc is not None, "balancer stuck"
            n[p][bc] += 1
            load[bc] += 1
            for e in rexp:
                rcnt[bc, e] += 1

    # quadratic-potential refinement
    WL = 0.3
    loads = load
    for _ in range(40):
        improved = False
        for p in groups:
            cs = cop[p]
            if len(cs) < 2:
                continue
            exps = rows_of(p)
            for cf in cs:
                for ct in cs:
                    if ct == cf:
                        continue
                    while n[p][cf] > 0 and loads[ct] < NAL * P + NGATH:
                        d = WL * 2.0 * (loads[ct] - loads[cf] + 1)
                        for e in exps:
                            d += 2.0 * (rcnt[ct, e] - rcnt[cf, e] + 1)
                        if d >= 0:
                            break
                        n[p][cf] -= 1
                        n[p][ct] += 1
                        loads[cf] -= 1
                        loads[ct] += 1
                        for e in exps:
                            rcnt[cf, e] -= 1
                            rcnt[ct, e] += 1
                        improved = True
        if not improved:
            break
    return n, rcnt


def _plan(te, tg):
    """Host routing plan.  te [N_TOK, 2] expert ids, tg [N_TOK, 2] gates.
    Returns (T, chunk readiness, per_core input dicts, token id tables)."""
    a = np.minimum(te[:, 0], te[:, 1])
    b = np.maximum(te[:, 0], te[:, 1])
    pid = (a * N_EXP + b).astype(np.int64)
    merged = a == b
    gsum = tg.sum(axis=1)

    direct_toks = [None] * NCORES
    gather_tok = np.ones(N_TOK, bool)
    for c in range(NCORES):
        e = MEXP[c]
        toks = np.where(merged & (a == e))[0][:NDIR]
        direct_toks[c] = toks
        gather_tok[toks] = False

    cnt_of = {}
    for p in np.unique(pid[gather_tok]):
        cnt_of[int(p)] = int(((pid == p) & gather_tok).sum())
    n, rcnt = _assign_counts(cnt_of)

    core_of = np.full(N_TOK, -1, np.int64)
    for p, npc in n.items():
        toks = np.where((pid == p) & gather_tok)[0]
        base = 0
        for c in range(NCORES):
            k = int(npc[c])
            core_of[toks[base:base + k]] = c
            base += k
    assert (core_of[gather_tok] >= 0).all()

    # per-core expert -> slot; MEXP[c] pinned to slot 3, brute-force over
    # the other 3 experts minimizing weighted aligned-pair overflow
    import itertools
    pair_caps = {}
    for sa, sb, k in APAIRS:
        pair_caps.setdefault((sa, sb), []).append(k)
    slots = []
    for c in range(NCORES):
        toks_c = np.where(core_of == c)[0]
        tcnt = {}
        movf = {}
        for t in toks_c:
            ea, eb = int(te[t, 0]), int(te[t, 1])
            if ea != eb:
                tcnt[(min(ea, eb), max(ea, eb))] = \
                    tcnt.get((min(ea, eb), max(ea, eb)), 0) + 1
            else:
                movf[ea] = movf.get(ea, 0) + 1
        rest = [e for e in BLOCKS[c] if e != MEXP[c]]
        best, bperm = None, None
        for perm3 in itertools.permutations(rest):
            perm = tuple(perm3) + (MEXP[c],)
            sl = {e: s for s, e in enumerate(perm)}
            exc_w, exc_tot = 0.0, 0
            for (ea, eb), cnt in tcnt.items():
                sa, sb = sorted((sl[ea], sl[eb]))
                cap = P * len(pair_caps.get((sa, sb), []))
                ex = max(0, cnt - cap)
                exc_tot += ex
                exc_w += ex * (1.0 + 4.0 * sb)
            for e, cnt in movf.items():
                exc_w += 0.5 * cnt * (1.0 + 2.0 * sl[e])
            score = exc_w + 1e6 * max(0, exc_tot - NGATH)
            if best is None or score < best:
                best, bperm = score, perm
        slots.append(list(bperm))

    cnt_cs = np.zeros((NCORES, NSLOT), np.int64)
    for c in range(NCORES):
        for s, e in enumerate(slots[c]):
            cnt_cs[c, s] = rcnt[c, e]
    T = np.maximum(1, -(-cnt_cs.max(axis=0) // P))
    off_t = np.concatenate([[0], np.cumsum(T)])
    NBs = int(T.sum())
    NB = NBs + 1
    NP = NB * P
    apairs = [(sa, sb, k, int(off_t[sa]) + k, int(off_t[sb]) + k)
              for sa, sb, k in APAIRS if k < T[sa] and k < T[sb]]

    per_core = []
    token_ids = []
    ready0_all = np.zeros((NCORES, len(CHUNKS)), np.int64)
    ready1_all = np.zeros((NCORES, len(CHUNKS)), np.int64)
    for c in range(NCORES):
        slot_of = {e: s for s, e in enumerate(slots[c])}
        toks_c = np.where(core_of == c)[0]
        by_sp = {}
        gather_list = []
        for t in toks_c:
            ea, eb = int(te[t, 0]), int(te[t, 1])
            if ea == eb:
                gather_list.append(t)
            else:
                sa, sb = sorted((slot_of[ea], slot_of[eb]))
                by_sp.setdefault((sa, sb), []).append(t)
        grow_flat = np.zeros(NP, np.float32)
        src_tok = np.full(NP, -1, np.int64)
        fill = np.zeros(NBs, np.int64)
        amap = np.full(NAL * P, -1, np.int64)
        alloc_order = sorted(range(len(apairs)),
                             key=lambda i: -apairs[i][2])
        for j in alloc_order:
            (sa, sb, k, ga, gb) = apairs[j]
            lst = by_sp.get((sa, sb), [])
            nal = min(len(lst), P)
            for p in range(nal):
                t = lst[p]
                amap[j * P + p] = t
                ea, eb = int(te[t, 0]), int(te[t, 1])
                if slot_of[ea] != sa:
                    ea, eb = eb, ea
                    g0, g1 = float(tg[t, 1]), float(tg[t, 0])
                else:
                    g0, g1 = float(tg[t, 0]), float(tg[t, 1])
                ra, rb = ga * P + p, gb * P + p
                grow_flat[ra] = g0
                src_tok[ra] = t
                grow_flat[rb] = g1
                src_tok[rb] = t
            by_sp[(sa, sb)] = lst[nal:]
            fill[ga] = max(fill[ga], nal)
            fill[gb] = max(fill[gb], nal)
        for lst in by_sp.values():
            gather_list.extend(lst)

        row_of = {}
        free = {int(g): list(range(int(fill[g]), P)) for g in range(NBs)}

        def place_row(s, t, g_val):
            for g in range(NBs):
                if off_t[s] <= g < off_t[s + 1] and free[g]:
                    p_ = free[g].pop(0)
                    r = g * P + p_
                    grow_flat[r] = g_val
                    src_tok[r] = t
                    row_of.setdefault(t, []).append(r)
                    return
            raise AssertionError("no free row slot")

        for t in gather_list:
            ea, eb = int(te[t, 0]), int(te[t, 1])
            if ea == eb:
                place_row(slot_of[ea], t, float(gsum[t]) * 0.5)
            else:
                place_row(slot_of[ea], t, float(tg[t, 0]))
                place_row(slot_of[eb], t, float(tg[t, 1]))

        for i, t in enumerate(direct_toks[c]):
            r = NBs * P + i
            grow_flat[r] = gsum[t]
            src_tok[r] = t

        # sort each token's rows by tile so r0 is the earlier-ready one
        for t in row_of:
            row_of[t].sort(key=lambda r: r // P)
        ready = np.array([row_of[t][-1] // P for t in gather_list], np.int64)
        order = np.argsort(ready, kind="stable")
        gl = np.array(gather_list, np.int64)[order]
        npad = NGATH - len(gl)
        assert npad >= 0, (c, len(gl))
        gtoks = np.concatenate([np.full(npad, -1, np.int64), gl])
        r0_flat = np.zeros(NGATH, np.int16)
        r1_flat = np.zeros(NGATH, np.int16)
        rd0 = np.zeros(NGATH, np.int64)
        rd1 = np.zeros(NGATH, np.int64)
        for pos, t in enumerate(gtoks):
            if t < 0:
                continue
            rs = row_of[t]
            r0_flat[pos] = rs[0]
            r1_flat[pos] = rs[-1] if len(rs) > 1 else rs[0]
            rd0[pos] = r0_flat[pos] // P
            rd1[pos] = r1_flat[pos] // P
        bounds = np.cumsum(CHUNKS)
        for ci in range(len(CHUNKS)):
            lo = bounds[ci] - CHUNKS[ci]
            ready0_all[c, ci] = rd0[lo:bounds[ci]].max()
            ready1_all[c, ci] = rd1[lo:bounds[ci]].max()
        dpad = np.full(NDIR, -1, np.int64)
        dpad[:len(direct_toks[c])] = direct_toks[c]
        token_ids.append((amap, dpad, gtoks))

        per_core.append(dict(
            grow=np.ascontiguousarray(grow_flat.reshape(NB, P).T),
            r0i=_pack16(r0_flat),
            r1i=_pack16(r1_flat),
            src_tok=src_tok,
            slot_experts=np.array(slots[c]),
        ))

    RT0 = np.maximum.accumulate(ready0_all.max(axis=0))
    RT1 = np.maximum.accumulate(ready1_all.max(axis=0))
    RT0 = np.minimum(RT0, RT1)
    max_real = max(int((tid[2] >= 0).sum()) for tid in token_ids)
    bounds = np.cumsum(CHUNKS)
    for ci in range(len(CHUNKS)):
        if bounds[ci] <= NGATH - max_real:
            RT1[ci] = -1                 # all-pad chunk: skip
    return T, RT0, RT1, per_core, token_ids


def _build_nc(T, RT0, RT1):
    NBs = int(T.sum())
    NB = NBs + 1
    off_tiles = np.concatenate([[0], np.cumsum(T)])

    nc = bacc.Bacc("TRN2", target_bir_lowering=False, debug=False,
                   num_devices=NCORES)

    xg = nc.dram_tensor("xg", [P, NB, KT, P], F16, kind="ExternalInput")
    wh = nc.dram_tensor("wh", [NSLOT, P, KT, D_OUT], F16,
                        kind="ExternalInput")
    grow = nc.dram_tensor("grow", [P, NB], F32, kind="ExternalInput")
    r0i = nc.dram_tensor("r0i", [P, NGATH // 16], I16, kind="ExternalInput")
    r1i = nc.dram_tensor("r1i", [P, NGATH // 16], I16, kind="ExternalInput")
    xwarm = nc.dram_tensor("xwarm", [P, P], F16, kind="ExternalInput")
    outT = nc.dram_tensor("outT", [P, (D_OUT // P) * NOUT], F16,
                          kind="ExternalOutput")

    # Pre-TileContext warmup: the first DMAGatherAnt triggers a ~15us Q7
    # extended-instruction library fetch; start it ASAP so it overlaps
    # the input DMAs and the first matmul tiles.
    warm_idx = nc.alloc_sbuf_tensor("warm_idx", [P, 8], I16)
    warm_dst = nc.alloc_sbuf_tensor("warm_dst", [P, P], F16)
    warm_sem = nc.alloc_semaphore("warm_set")
    warm_dma = nc.alloc_semaphore("warm_dma")
    nc.gpsimd.memset(warm_idx.ap(), 0).then_inc(warm_sem, 1)
    nc.gpsimd.wait_ge(warm_sem, 1)
    nc.gpsimd.dma_gather(
        warm_dst.ap().rearrange("p (a b) -> p a b", a=1),
        xwarm[:].rearrange("n (a b) -> (n a) b", b=P),
        warm_idx.ap(), num_idxs=P, num_idxs_reg=P, elem_size=P,
        transpose=True).then_inc(warm_dma, 16)

    # chunk schedule: (chunk idx, token base, size, r0-ready, r1-ready);
    # RT1 == -1 marks an all-pad chunk (not emitted)
    chunk_q = []
    base = 0
    for ci, ch in enumerate(CHUNKS):
        if int(RT1[ci]) >= 0:
            chunk_q.append([ci, base, ch, int(RT0[ci]), int(RT1[ci])])
        base += ch
    apairs = [(j, int(off_tiles[sa]) + k, int(off_tiles[sb]) + k)
              for j, (sa, sb, k) in enumerate(APAIRS)
              if k < T[sa] and k < T[sb]]

    with tile.TileContext(nc) as tc:
        with (
            tc.tile_pool(name="const", bufs=1) as kpool,
            tc.tile_pool(name="w", bufs=4) as wpool,
            tc.tile_pool(name="xT", bufs=1) as xpool,
            tc.tile_pool(name="y", bufs=1) as ypool,
            tc.tile_pool(name="cmb", bufs=8) as cpool,
            tc.tile_pool(name="ot", bufs=3) as opool,
            tc.tile_pool(name="ps", bufs=4, space="PSUM") as ppool,
        ):
            x_t = xpool.tile([P, NB, KT, P], F16)
            w_ts = [wpool.tile([P, KT, D_OUT], F16, tag="w",
                               name="w%d" % s_) for s_ in range(NSLOT)]
            grow_t = kpool.tile([P, NB], F32)
            r0_t = kpool.tile([P, NGATH // 16], I16)
            r1_t = kpool.tile([P, NGATH // 16], I16)
            # One input queue (sync), strict deadline order: interleave
            # weight chunks and x slots so neither starves the other
            # (per-packet round-robin across queues lets 16KB weight
            # packets take 8x the bandwidth of 2KB x packets otherwise).
            def xdma(lo, hi):
                lo, hi = int(lo), int(min(hi, NB))
                if lo < hi:
                    nc.sync.dma_start(x_t[:, lo:hi], xg[:, lo:hi])

            def wdma(s, klo, khi):
                nc.sync.dma_start(w_ts[s][:, klo:khi], wh[s, :, klo:khi])

            o1, o2, o3 = int(off_tiles[1]), int(off_tiles[2]), int(off_tiles[3])
            wdma(0, 0, 2)
            xdma(0, 2)
            nc.sync.dma_start(grow_t[:], grow[:])
            xdma(2, o1)
            wdma(0, 2, KT)
            xdma(o1, o1 + 2)
            wdma(1, 0, 4)
            xdma(o1 + 2, o2)
            wdma(1, 4, KT)
            nc.sync.dma_start(r0_t[:], r0i[:])
            nc.sync.dma_start(r1_t[:], r1i[:])
            xdma(o2, o2 + 2)
            wdma(2, 0, 4)
            xdma(o2 + 2, o3)
            wdma(2, 4, KT)
            xdma(o3, o3 + 2)
            wdma(3, 0, 4)
            xdma(o3 + 2, NBs)
            wdma(3, 4, KT)
            xdma(NBs, NB)

            y_t = ypool.tile([P, NB, D_OUT], F16)

            gtiles = {}                  # ci -> (g0, g1, cbase, csize)

            def emit_gather(ci, cbase, csize, rt, which):
                dst = cpool.tile([P, D_OUT // P, csize], F16,
                                 tag="g%d_%d" % (which, csize), bufs=2)
                ridx = r0_t if which == 0 else r1_t
                nc.gpsimd.dma_gather(
                    dst[:], y_t[:, :rt + 1, :],
                    ridx[:, cbase // 16:(cbase + csize) // 16],
                    num_idxs=csize, num_idxs_reg=csize,
                    elem_size=D_OUT, transpose=True,
                    sbuf_tokens_per_rank=P,
                    sbuf_free_dim_per_rank=D_OUT * 2,
                )
                return dst

            gbase = (NAL * P + NDIR) * (D_OUT // P)

            def emit_chunk_out(ci, cbase, csize):
                g0, g1 = gtiles[ci]
                ot = opool.tile([P, D_OUT // P, csize], F16,
                                tag="ots_%d" % csize, bufs=2)
                nc.vector.tensor_add(out=ot[:], in0=g0[:], in1=g1[:])
                nc.gpsimd.dma_start(
                    outT[:, gbase + cbase * (D_OUT // P):
                         gbase + (cbase + csize) * (D_OUT // P)],
                    ot[:].rearrange("p a b -> p (a b)"))

            qi = 0          # next chunk awaiting full emission (r1 + out)
            q0 = 0          # next chunk awaiting r0-gather emission

            def pump_chunks(g):
                nonlocal qi, q0
                while q0 < len(chunk_q) and chunk_q[q0][3] <= g:
                    ci, cbase, csize, rt0, rt1 = chunk_q[q0]
                    gtiles[ci] = [emit_gather(ci, cbase, csize, rt0, 0),
                                  None]
                    q0 += 1
                while qi < len(chunk_q) and chunk_q[qi][4] <= g:
                    ci, cbase, csize, rt0, rt1 = chunk_q[qi]
                    if qi >= q0:         # r0 not yet emitted (rt0 race)
                        gtiles[ci] = [emit_gather(ci, cbase, csize, rt1, 0),
                                      None]
                        q0 = qi + 1
                    gtiles[ci][1] = emit_gather(ci, cbase, csize, rt1, 1)
                    emit_chunk_out(ci, cbase, csize)
                    qi += 1

            for s in range(NSLOT + 1):
                w_t = w_ts[min(s, NSLOT - 1)]
                lo = int(off_tiles[s]) if s < NSLOT else NBs
                hi = int(off_tiles[s + 1]) if s < NSLOT else NB
                for g in range(lo, hi):
                    ps0 = ppool.tile([P, 512], F32, tag="ps")
                    ps1 = ppool.tile([P, 512], F32, tag="ps")
                    for kk in range(KT):
                        lhsT = x_t[:, g, kk, :]
                        nc.tensor.matmul(ps0[:], lhsT, w_t[:, kk, 0:512],
                                         start=(kk == 0), stop=(kk == KT - 1))
                        nc.tensor.matmul(ps1[:], lhsT, w_t[:, kk, 512:1024],
                                         start=(kk == 0), stop=(kk == KT - 1))
                    gsc = grow_t[:, g:g + 1]
                    nc.scalar.activation(y_t[:, g, 0:512], ps0[:],
                                         mybir.ActivationFunctionType.Copy,
                                         scale=gsc)
                    nc.scalar.activation(y_t[:, g, 512:1024], ps1[:],
                                         mybir.ActivationFunctionType.Copy,
                                         scale=gsc)
                    for j, ga, gb in apairs:
                        if max(ga, gb) != g:
                            continue
                        ot = opool.tile([P, D_OUT], F16, tag="al")
                        nc.vector.tensor_add(out=ot[:], in0=y_t[:, ga, :],
                                             in1=y_t[:, gb, :])
                        nc.gpsimd.dma_start(
                            outT[:, j * P * (D_OUT // P):
                                 (j + 1) * P * (D_OUT // P)],
                            ot[:])
                    if g < NBs:
                        pump_chunks(g)
            pump_chunks(NBs)
            # merged direct-out tile: evicted y rows ARE output rows
            nc.scalar.dma_start(
                outT[:, NAL * P * (D_OUT // P):
                     (NAL * P + NDIR) * (D_OUT // P)],
                y_t[:, NB - 1, :])

    nc.compile()
    return nc


def _prep(inputs):
    x = np.asarray(inputs["input"], np.float32)
    w = np.asarray(inputs["weight"], np.float32)
    k = int(np.asarray(inputs["k"]))
    assert k == TOPK
    sei = np.asarray(inputs["sorted_expert_indices"]).astype(np.int64)
    ssi = np.asarray(inputs["sorted_scattered_indices"]).astype(np.int64)
    gates = np.asarray(inputs["gates"], np.float32)

    tok = ssi // k
    g_row = gates.reshape(-1)[ssi]
    order_by_tok = np.argsort(tok, kind="stable")
    te = sei[order_by_tok].reshape(N_TOK, TOPK)
    tg = g_row[order_by_tok].reshape(N_TOK, TOPK)

    T, RT0, RT1, per_core, token_ids = _plan(te, tg)
    NB = int(T.sum()) + 1
    NP = NB * P

    xh = x.astype(np.float16)
    whp = np.ascontiguousarray(
        w.reshape(N_EXP, KT, P, D_OUT).transpose(0, 2, 1, 3)
    ).astype(np.float16)               # [E, P, KT, D_OUT]
    xwarm = np.zeros((P, P), np.float16)

    in_maps = []
    for c in range(NCORES):
        pc = per_core[c]
        src = pc.pop("src_tok")
        se = pc.pop("slot_experts")
        A = np.zeros((NP, D_IN), np.float16)
        m = src >= 0
        A[m] = xh[src[m]]
        AT = np.ascontiguousarray(A.T)                  # [D_IN, NP]
        xgc = np.ascontiguousarray(
            AT.reshape(KT, P, NB, P).transpose(1, 2, 0, 3))
        in_maps.append(dict(
            xg=xgc,
            wh=np.ascontiguousarray(whp[se]),
            grow=pc["grow"], r0i=pc["r0i"], r1i=pc["r1i"],
            xwarm=xwarm,
        ))
    return T, RT0, RT1, in_maps, token_ids


def _run(inputs, trace=False, trace_kwargs=None):
    T, RT0, RT1, in_maps, token_ids = _prep(inputs)
    nc = _build_nc(T, RT0, RT1)
    res = run_bass_kernel_spmd(
        nc, in_maps, core_ids=list(range(NCORES)), trace=trace,
        **(trace_kwargs or {}),
    )
    out = np.zeros((N_TOK, D_OUT), np.float32)
    for c in range(NCORES):
        oT = res.results[c]["outT"]                      # [P, 8*NOUT]
        amap, dtoks, gtoks = token_ids[c]
        ablk = oT[:, :(NAL * P + NDIR) * (D_OUT // P)]
        rows = ablk.reshape(P, NAL + 1, D_OUT).transpose(1, 0, 2) \
                   .reshape((NAL + 1) * P, D_OUT)
        ids = np.concatenate([amap, dtoks])
        m = ids >= 0
        out[ids[m]] = rows[m]
        gbase = (NAL * P + NDIR) * (D_OUT // P)
        base = 0
        for ch in CHUNKS:
            blk = oT[:, gbase + base * (D_OUT // P):
                     gbase + (base + ch) * (D_OUT // P)]
            rws = blk.reshape(P, D_OUT // P, ch).transpose(2, 1, 0)
            ids = gtoks[base:base + ch]
            m = ids >= 0
            out[ids[m]] = rws.reshape(ch, D_OUT)[m]
            base += ch
    return out, res


def kernel(**inputs) -> np.ndarray:
    out, _ = _run(inputs, trace=bool(int(os.environ.get("KERNEL_TRACE", "0"))))
    return out
